# revision 36
# baseline (speedup 1.0000x reference)
"""BottAttention Trainium2 kernel (v4).

Reference computation (per batch b):
    qkv = x @ W_qkv                       # [N, 3*H*D]
    q,k,v per head h (D=64)
    S = q @ k.T * D**-0.5                 # [N, N]
    P = softmax(S, axis=-1) + reg[h]      # post-softmax learned bias
    o = P @ v                             # [N, D]
    out = concat_h(o) @ W_proj + b_proj   # [N, C]

reg as generated is UNIFORM (ones/N): its contribution to out is the
per-batch row-constant  c[b] = r * (x[b].sum(0) @ W_v) @ W_proj,  added
on the host (verified at runtime; general numpy fallback otherwise).
The device computes pure softmax attention, all-bf16 storage, fp32 PSUM.

Sharding: 2 batch-groups x 4 head-groups over 8 cores.  Core c handles
batches 4*(c//4)..+4 and heads 4*(c%4)..+4; host sums the 4 head-group
partial projections, transposes, adds b_proj + c[b].

Device dataflow per (core, batch):
    qT,kT  = (W_qk tiles).T @ xT          # [wcol, tok]
    v      = (xT tiles).T @ W_v           # [tok, vcol] (+ ones col)
    ST     = kT.T @ qT                    # [j, i]; 2 heads concurrently in
                                          # disjoint PE row groups (0/64)
    est    = exp(ST * scale)              # ACT, psum -> sbuf bf16
    avT    = [v | 1].T @ est              # rows 0-63 (attn@v).T, row 64 rowsum
    attnT  = avT * bcast(1/rowsum)        # fast recip + gpsimd bcast
    outT   = W_proj_rows.T @ attnT        # partial [C, i], bf16 out

All DRAM<->SBUF layouts are identity-mapped to the SBUF tiles (multi-KB
contiguous runs per partition, one DMA per tensor/batch).  Emission
pipelines batches: phase A of b+1 is emitted between attention(b) and
proj(b, 1), filling the ACT-bound softmax stretch and keeping the PE warm
across batch boundaries.
"""

import os
import numpy as np
import ml_dtypes
from contextlib import ExitStack

import concourse.bass as bass
import concourse.bacc as bacc
import concourse.tile as tile
from concourse import mybir
from concourse.bass_utils import run_bass_kernel_spmd

FP32 = mybir.dt.float32
BF16 = mybir.dt.bfloat16

# Problem dims (hardcoded per contest contract)
B, H, N, C, D = 8, 16, 1024, 1024, 64
SCALE = D ** -0.5

# Per-core shard dims
NB = 4          # batches per core
NH = 4          # heads per core
KC = C // 128   # contraction tiles for dim C
TT = N // 128   # token tiles
NBLK = 512      # matmul moving-dim block
IB = N // NBLK  # i blocks

N_CORES = 8


def build_program():
    nc = bacc.Bacc("TRN2", debug=False, enable_asserts=False, num_devices=1)

    # identity-mapped layouts: [128 partitions, ...contiguous free dims]
    xb = nc.dram_tensor("xb", [NB, 128, KC, N], BF16, kind="ExternalInput").ap()
    wqk = nc.dram_tensor("wqk", [128, 4, KC, 128], BF16, kind="ExternalInput").ap()
    wv = nc.dram_tensor("wv", [128, KC, NH * D], BF16, kind="ExternalInput").ap()
    wp = nc.dram_tensor("wp", [2, 128, C], BF16, kind="ExternalInput").ap()
    outp = nc.dram_tensor("outp", [NB, IB, 128, KC, NBLK], BF16,
                          kind="ExternalOutput").ap()

    EXPF = mybir.ActivationFunctionType.Exp

    with (
        nc.allow_low_precision(reason="bf16 attention path is intentional"),
        tile.TileContext(nc) as tc,
        ExitStack() as top,
    ):
        # ---- input DMAs first (x batch 0, then weights, in need order) ----
        xpool = top.enter_context(tc.tile_pool(name="xt", bufs=2))
        wpool = top.enter_context(tc.tile_pool(name="wq", bufs=1))

        def dma_x(b, split=False):
            t = xpool.tile([128, KC, N], BF16, tag="xt")
            if split:
                # two halves so batch 0's first qk matmuls start sooner
                nc.sync.dma_start(t[:, 0:KC // 2, :], xb[b, :, 0:KC // 2, :])
                nc.sync.dma_start(t[:, KC // 2:KC, :], xb[b, :, KC // 2:KC, :])
            else:
                nc.sync.dma_start(t[:], xb[b])
            return t

        # first qk group needs wqk[m=0] + x k-tiles: wqk via the scalar
        # HWDGE (in m-need order: 0, 2, 1, 3), x via sync, so the two
        # transfer in parallel right after the framework preamble
        wqk_sb = wpool.tile([128, 4, KC, 128], BF16, tag="wqk")
        nc.scalar.dma_start(wqk_sb[:, 0], wqk[:, 0])
        xk0 = dma_x(0, split=True)
        nc.scalar.dma_start(wqk_sb[:, 2], wqk[:, 2])
        nc.scalar.dma_start(wqk_sb[:, 1], wqk[:, 1])
        nc.scalar.dma_start(wqk_sb[:, 3], wqk[:, 3])
        wv_sb = wpool.tile([128, KC, NH * D], BF16, tag="wv")
        nc.scalar.dma_start(wv_sb[:], wv)
        persist = top.enter_context(tc.tile_pool(name="persist", bufs=1))
        wp_sb = [persist.tile([128, C], BF16, tag=f"wp{k}", name=f"wp{k}")
                 for k in range(2)]
        for k in range(2):
            nc.sync.dma_start(wp_sb[k][:], wp[k])

        # qk_sb[b][m]: m=0,1 -> qT head pairs; m=2,3 -> kT pairs
        qk_sb = [
            [persist.tile([128, N], BF16, tag=f"qk{b}_{m}", name=f"qk{b}_{m}")
             for m in range(4)]
            for b in range(NB)
        ]
        # v_sb[t]: [128, NB, NH, 65] bf16; 64 v cols + ones col (rowsum)
        v_sb = [
            persist.tile([128, NB, NH, 65], BF16, tag=f"v{t}", name=f"v{t}")
            for t in range(TT)
        ]
        # att_sb[b][hp]: normalized (attn@v).T for head pair hp
        att_sb = [
            [persist.tile([128, N], BF16, tag=f"at{b}_{k}", name=f"at{b}_{k}")
             for k in range(2)]
            for b in range(NB)
        ]

        epool = top.enter_context(tc.tile_pool(name="est", bufs=12))
        small = top.enter_context(tc.tile_pool(name="sm", bufs=2))
        outs_pool = top.enter_context(tc.tile_pool(name="outs", bufs=2))
        psA = top.enter_context(tc.tile_pool(name="psA", bufs=2, space="PSUM"))
        psS = top.enter_context(tc.tile_pool(name="psS", bufs=1, space="PSUM"))
        psAv = top.enter_context(tc.tile_pool(name="psAv", bufs=2, space="PSUM"))

        # PE warmup: cheap matmuls so HAM un-throttles while x DMA lands
        warm_src = wpool.tile([1, NBLK], BF16, tag="warm_src")
        nc.vector.memset(warm_src[:], 1.0)
        wps = psA.tile([1, NBLK], FP32, tag="mm")
        for w in range(6):
            nc.tensor.matmul(
                wps[:], lhsT=warm_src[0:1, 0:1], rhs=warm_src[:],
                start=True, stop=True,
            )

        def qk_mm(b, xk, m):
            for tb in range(IB):
                ps = psA.tile([128, NBLK], FP32, tag="mm")
                for k in range(KC):
                    nc.tensor.matmul(
                        ps[:],
                        lhsT=wqk_sb[:, m, k, :],
                        rhs=xk[:, k, tb * NBLK:(tb + 1) * NBLK],
                        start=(k == 0),
                        stop=(k == KC - 1),
                    )
                nc.any.tensor_copy(
                    qk_sb[b][m][:, tb * NBLK:(tb + 1) * NBLK], ps[:]
                )

        def phase_a(b, xk):
            # hp0's q/k first, then v, then hp1's: attention(b, 0, 0) can
            # start after m=0,2 and v
            qk_mm(b, xk, 0)
            qk_mm(b, xk, 2)
            for tt in range(TT):
                ps = psA.tile([128, NH * D], FP32, tag="mm")
                for k in range(KC):
                    nc.tensor.matmul(
                        ps[:],
                        lhsT=xk[:, k, tt * 128:(tt + 1) * 128],
                        rhs=wv_sb[:, k, :],
                        start=(k == 0),
                        stop=(k == KC - 1),
                    )
                vd = v_sb[tt][:]
                nc.vector.tensor_copy(
                    vd[:, b, :, 0:64], ps[:].rearrange("p (h s) -> p h s", s=64)
                )
                nc.vector.memset(vd[:, b, :, 64:65], 1.0)
            qk_mm(b, xk, 1)
            qk_mm(b, xk, 3)

        def attention(b, ib, hp):
            qt = qk_sb[b][hp]
            kt = qk_sb[b][2 + hp]
            # scores for TWO j-tiles share one 4-bank psum tile so each
            # exp ACTIVATE covers 2048 elems/partition (halves ACT call
            # overhead, the phase-B bottleneck)
            ests = []
            for jp in range(TT // 2):
                ps = psS.tile([128, 2, 2, NBLK], FP32, tag="st")
                for jj in range(2):
                    j = 2 * jp + jj
                    for h2 in range(2):
                        po = h2 * 64
                        nc.tensor.matmul(
                            ps[:, jj, h2, :],
                            lhsT=kt[po:po + 64, j * 128:(j + 1) * 128],
                            rhs=qt[po:po + 64, ib * NBLK:(ib + 1) * NBLK],
                            start=True,
                            stop=True,
                        )
                e = epool.tile([128, 2, 2, NBLK], BF16, tag="est")
                nc.scalar.activation(e[:], ps[:], EXPF, scale=SCALE)
                ests.append(e)
            for h2 in range(2):
                h = hp * 2 + h2
                av = psAv.tile([128, NBLK], FP32, tag="av")
                for j in range(TT):
                    jp, jj = divmod(j, 2)
                    nc.tensor.matmul(
                        av[0:65, :],
                        lhsT=v_sb[j][:, b, h, :],
                        rhs=ests[jp][:, jj, h2, :],
                        start=(j == 0),
                        stop=(j == TT - 1),
                    )
                # custom-DVE recip mis-reads PSUM; stage via SBUF
                rsum = small.tile([1, NBLK], FP32, tag="rsum")
                nc.vector.tensor_copy(rsum[:], av[64:65, :])
                rcp = small.tile([1, NBLK], FP32, tag="rcp")
                nc.vector.reciprocal_approx_fast(rcp[:], rsum[:])
                rbc = small.tile([64, NBLK], FP32, tag="rbc")
                nc.gpsimd.partition_broadcast(rbc[:], rcp[:], channels=64)
                nc.vector.tensor_mul(
                    att_sb[b][hp][h2 * 64:(h2 + 1) * 64,
                                  ib * NBLK:(ib + 1) * NBLK],
                    av[0:64, :],
                    rbc[:],
                )

        def proj(b, ib):
            # stage ct-tiles; identity-mapped DMA per half so the last
            # half's DMA is the only exposed tail
            ot = outs_pool.tile([128, KC, NBLK], BF16, tag="ot")
            for ct in range(KC):
                ps = psA.tile([128, NBLK], FP32, tag="mm")
                for k in range(2):
                    nc.tensor.matmul(
                        ps[:],
                        lhsT=wp_sb[k][:, ct * 128:(ct + 1) * 128],
                        rhs=att_sb[b][k][:, ib * NBLK:(ib + 1) * NBLK],
                        start=(k == 0),
                        stop=(k == 1),
                    )
                nc.vector.tensor_copy(ot[:, ct, :], ps[:])
                if ct == KC // 2 - 1:
                    nc.sync.dma_start(
                        outp[b, ib, :, 0:KC // 2, :], ot[:, 0:KC // 2, :]
                    )
            nc.sync.dma_start(
                outp[b, ib, :, KC // 2:KC, :], ot[:, KC // 2:KC, :]
            )

        phase_a(0, xk0)
        for b in range(NB):
            if b + 1 < NB:
                xk_next = dma_x(b + 1)
            for ib in range(IB):
                for hp in range(NH // 2):
                    attention(b, ib, hp)
                if ib == 0:
                    proj(b, 0)
            # phase A of b+1 fills batch b's ACT-bound PE gaps and keeps the
            # PE warm across the batch boundary; proj(b, 1) slots in behind
            if b + 1 < NB:
                phase_a(b + 1, xk_next)
            proj(b, 1)

    nc.compile()
    return nc


_NC = None


def _get_program():
    global _NC
    if _NC is None:
        _NC = build_program()
    return _NC


def make_in_maps(x, W_qkv):
    """Host-side sharding: per-core input dicts (all bf16, identity layouts)."""
    x = np.asarray(x, dtype=np.float32)
    W_qkv = np.asarray(W_qkv, dtype=np.float32)
    in_maps = []
    for c in range(N_CORES):
        bg, hg = divmod(c, 4)
        # [NB, 128, KC, N]: xb[b, p, k, n] = x[b, n, k*128+p]
        xT_c = x[bg * NB:(bg + 1) * NB].transpose(0, 2, 1)   # [NB, C, N]
        xb_c = np.ascontiguousarray(
            xT_c.reshape(NB, KC, 128, N).transpose(0, 2, 1, 3)
        ).astype(ml_dtypes.bfloat16)
        q_cols = W_qkv[:, hg * NH * D:(hg + 1) * NH * D]
        k_cols = W_qkv[:, H * D + hg * NH * D:H * D + (hg + 1) * NH * D]
        wqk_c = np.concatenate([q_cols, k_cols], axis=1)     # [C, 512]
        # [128, 4, KC, 128]: wqk[p, m, k, c] = wqk_c[k*128+p, m*128+c]
        wqk_c = np.ascontiguousarray(
            wqk_c.reshape(KC, 128, 4, 128).transpose(1, 2, 0, 3)
        ).astype(ml_dtypes.bfloat16)
        wv_c = W_qkv[:, 2 * H * D + hg * NH * D:2 * H * D + (hg + 1) * NH * D]
        wv_c = np.ascontiguousarray(
            wv_c.reshape(KC, 128, NH * D).transpose(1, 0, 2)
        ).astype(ml_dtypes.bfloat16)
        in_maps.append({
            "xb": xb_c,
            "wqk": wqk_c,
            "wv": wv_c,
        })
    return in_maps


def reg_row_constant(x, W_qkv, reg, W_proj):
    """Contribution of the post-softmax reg bias to the output.

    Uniform reg (the generated input): rank-1 per batch -> [B, 1, C].
    Non-uniform reg: full numpy fallback -> [B, N, C].
    """
    x = np.asarray(x, dtype=np.float32)
    W_qkv = np.asarray(W_qkv, dtype=np.float32)
    W_proj = np.asarray(W_proj, dtype=np.float32)
    reg = np.asarray(reg, dtype=np.float32)
    W_v = W_qkv[:, 2 * H * D:]
    if reg.max() == reg.min():
        r = float(reg.flat[0])
        xsum = x.sum(axis=1)                      # [B, C]
        return (r * (xsum @ W_v) @ W_proj)[:, None, :]
    v = (x @ W_v).reshape(B, N, H, D)
    regv = np.einsum("hnm,bmhd->bnhd", reg[0], v)
    return regv.reshape(B, N, H * D) @ W_proj


def assemble_output(results, corr, b_proj):
    b_proj = np.asarray(b_proj, dtype=np.float32)
    out = np.empty((B, N, C), dtype=np.float32)
    for b in range(B):
        bg, bl = divmod(b, NB)
        accT = None
        for hg in range(4):
            o = np.asarray(results[bg * 4 + hg]["outp"][bl])  # [IB,128,KC,NBLK]
            oT = o.transpose(2, 1, 0, 3).reshape(C, N).astype(np.float32)
            accT = oT if accT is None else accT + oT
        cb = corr[b] if corr.shape[1] > 1 else corr[b, 0]
        out[b] = accT.T + (cb + b_proj)
    return out


def kernel(x, W_qkv, reg, W_proj, b_proj, trace=None):
    if trace is None:
        trace = bool(int(os.environ.get("KERNEL_TRACE", "0")))
    nc = _get_program()
    in_maps = make_in_maps(x, W_qkv)
    W_proj = np.asarray(W_proj, dtype=np.float32)
    for c in range(N_CORES):
        hg = c % 4
        in_maps[c]["wp"] = np.ascontiguousarray(
            W_proj[hg * NH * D:(hg + 1) * NH * D, :].reshape(2, 128, C)
        ).astype(ml_dtypes.bfloat16)
    corr = reg_row_constant(x, W_qkv, reg, W_proj)
    try:
        res = run_bass_kernel_spmd(
            nc, in_maps, core_ids=list(range(N_CORES)), trace=trace
        )
    except ModuleNotFoundError:
        # profiling hook unavailable in this environment: run without trace
        res = run_bass_kernel_spmd(
            nc, in_maps, core_ids=list(range(N_CORES)), trace=False
        )
    kernel.last_results = res
    return assemble_output(res.results, corr, b_proj)


# revision 37
# speedup vs baseline: 1.0872x; 1.0872x over previous
"""BottAttention Trainium2 kernel (v4).

Reference computation (per batch b):
    qkv = x @ W_qkv                       # [N, 3*H*D]
    q,k,v per head h (D=64)
    S = q @ k.T * D**-0.5                 # [N, N]
    P = softmax(S, axis=-1) + reg[h]      # post-softmax learned bias
    o = P @ v                             # [N, D]
    out = concat_h(o) @ W_proj + b_proj   # [N, C]

reg as generated is UNIFORM (ones/N): its contribution to out is the
per-batch row-constant  c[b] = r * (x[b].sum(0) @ W_v) @ W_proj,  added
on the host (verified at runtime; general numpy fallback otherwise).
The device computes pure softmax attention, all-bf16 storage, fp32 PSUM.

Sharding: 2 batch-groups x 4 head-groups over 8 cores.  Core c handles
batches 4*(c//4)..+4 and heads 4*(c%4)..+4; host sums the 4 head-group
partial projections, transposes, adds b_proj + c[b].

Device dataflow per (core, batch):
    qT,kT  = (W_qk tiles).T @ xT          # [wcol, tok]
    v      = (xT tiles).T @ W_v           # [tok, vcol] (+ ones col)
    ST     = kT.T @ qT                    # [j, i]; 2 heads concurrently in
                                          # disjoint PE row groups (0/64)
    est    = exp(ST * scale)              # ACT, psum -> sbuf bf16
    avT    = [v | 1].T @ est              # rows 0-63 (attn@v).T, row 64 rowsum
    attnT  = avT * bcast(1/rowsum)        # fast recip + gpsimd bcast
    outT   = W_proj_rows.T @ attnT        # partial [C, i], bf16 out

All DRAM<->SBUF layouts are identity-mapped to the SBUF tiles (multi-KB
contiguous runs per partition, one DMA per tensor/batch).  Emission
pipelines batches: phase A of b+1 is emitted between attention(b) and
proj(b, 1), filling the ACT-bound softmax stretch and keeping the PE warm
across batch boundaries.
"""

import os
import numpy as np
import ml_dtypes
from contextlib import ExitStack

import concourse.bass as bass
import concourse.bacc as bacc
import concourse.tile as tile
from concourse import mybir
from concourse.bass_utils import run_bass_kernel_spmd

FP32 = mybir.dt.float32
BF16 = mybir.dt.bfloat16

# Problem dims (hardcoded per contest contract)
B, H, N, C, D = 8, 16, 1024, 1024, 64
SCALE = D ** -0.5

# Per-core shard dims
NB = 4          # batches per core
NH = 4          # heads per core
KC = C // 128   # contraction tiles for dim C
TT = N // 128   # token tiles
NBLK = 512      # matmul moving-dim block
IB = N // NBLK  # i blocks

N_CORES = 8


def build_program():
    nc = bacc.Bacc("TRN2", debug=False, enable_asserts=False, num_devices=1)

    # identity-mapped layouts: [128 partitions, ...contiguous free dims]
    xb = nc.dram_tensor("xb", [NB, 128, KC, N], BF16, kind="ExternalInput").ap()
    wqk = nc.dram_tensor("wqk", [128, 4, KC, 128], BF16, kind="ExternalInput").ap()
    wv = nc.dram_tensor("wv", [128, KC, NH * D], BF16, kind="ExternalInput").ap()
    wp = nc.dram_tensor("wp", [2, 128, C], BF16, kind="ExternalInput").ap()
    outp = nc.dram_tensor("outp", [NB, IB, 128, KC, NBLK], BF16,
                          kind="ExternalOutput").ap()

    EXPF = mybir.ActivationFunctionType.Exp

    with (
        nc.allow_low_precision(reason="bf16 attention path is intentional"),
        tile.TileContext(nc) as tc,
        ExitStack() as top,
    ):
        # ---- input DMAs first (x batch 0, then weights, in need order) ----
        xpool = top.enter_context(tc.tile_pool(name="xt", bufs=2))
        wpool = top.enter_context(tc.tile_pool(name="wq", bufs=1))

        def dma_x(b, split=False):
            t = xpool.tile([128, KC, N], BF16, tag="xt")
            if split:
                # two halves so batch 0's first qk matmuls start sooner
                nc.sync.dma_start(t[:, 0:KC // 2, :], xb[b, :, 0:KC // 2, :])
                nc.sync.dma_start(t[:, KC // 2:KC, :], xb[b, :, KC // 2:KC, :])
            else:
                nc.sync.dma_start(t[:], xb[b])
            return t

        # first qk group needs wqk[m=0] + x k-tiles: wqk via the scalar
        # HWDGE (in m-need order: 0, 2, 1, 3), x via sync, so the two
        # transfer in parallel right after the framework preamble
        wqk_sb = wpool.tile([128, 4, KC, 128], BF16, tag="wqk")
        nc.scalar.dma_start(wqk_sb[:, 0], wqk[:, 0])
        xk0 = dma_x(0, split=True)
        nc.scalar.dma_start(wqk_sb[:, 2], wqk[:, 2])
        nc.scalar.dma_start(wqk_sb[:, 1], wqk[:, 1])
        nc.scalar.dma_start(wqk_sb[:, 3], wqk[:, 3])
        wv_sb = wpool.tile([128, KC, NH * D], BF16, tag="wv")
        nc.scalar.dma_start(wv_sb[:], wv)
        persist = top.enter_context(tc.tile_pool(name="persist", bufs=1))
        wp_sb = [persist.tile([128, C], BF16, tag=f"wp{k}", name=f"wp{k}")
                 for k in range(2)]
        for k in range(2):
            nc.sync.dma_start(wp_sb[k][:], wp[k])

        # qk_sb[b][m]: m=0,1 -> qT head pairs; m=2,3 -> kT pairs
        qk_sb = [
            [persist.tile([128, N], BF16, tag=f"qk{b}_{m}", name=f"qk{b}_{m}")
             for m in range(4)]
            for b in range(NB)
        ]
        # v_sb[t]: [128, NB, NH, 65] bf16; 64 v cols + ones col (rowsum)
        v_sb = [
            persist.tile([128, NB, NH, 65], BF16, tag=f"v{t}", name=f"v{t}")
            for t in range(TT)
        ]
        # att_sb[b][hp]: normalized (attn@v).T for head pair hp
        att_sb = [
            [persist.tile([128, N], BF16, tag=f"at{b}_{k}", name=f"at{b}_{k}")
             for k in range(2)]
            for b in range(NB)
        ]

        epool = top.enter_context(tc.tile_pool(name="est", bufs=12))
        small = top.enter_context(tc.tile_pool(name="sm", bufs=2))
        outs_pool = top.enter_context(tc.tile_pool(name="outs", bufs=2))
        psA = top.enter_context(tc.tile_pool(name="psA", bufs=2, space="PSUM"))
        psS = top.enter_context(tc.tile_pool(name="psS", bufs=2, space="PSUM"))
        psAv = top.enter_context(tc.tile_pool(name="psAv", bufs=2, space="PSUM"))

        # PE warmup: cheap matmuls so HAM un-throttles while x DMA lands
        warm_src = wpool.tile([1, NBLK], BF16, tag="warm_src")
        nc.vector.memset(warm_src[:], 1.0)
        wps = psA.tile([1, NBLK], FP32, tag="mm")
        for w in range(6):
            nc.tensor.matmul(
                wps[:], lhsT=warm_src[0:1, 0:1], rhs=warm_src[:],
                start=True, stop=True,
            )

        def qk_mm(b, xk, m):
            for tb in range(IB):
                ps = psA.tile([128, NBLK], FP32, tag="mm")
                for k in range(KC):
                    nc.tensor.matmul(
                        ps[:],
                        lhsT=wqk_sb[:, m, k, :],
                        rhs=xk[:, k, tb * NBLK:(tb + 1) * NBLK],
                        start=(k == 0),
                        stop=(k == KC - 1),
                    )
                nc.any.tensor_copy(
                    qk_sb[b][m][:, tb * NBLK:(tb + 1) * NBLK], ps[:]
                )

        def phase_a(b, xk):
            # hp0's q/k first, then v, then hp1's: attention(b, 0, 0) can
            # start after m=0,2 and v
            qk_mm(b, xk, 0)
            qk_mm(b, xk, 2)
            for tt in range(TT):
                ps = psA.tile([128, NH * D], FP32, tag="mm")
                for k in range(KC):
                    nc.tensor.matmul(
                        ps[:],
                        lhsT=xk[:, k, tt * 128:(tt + 1) * 128],
                        rhs=wv_sb[:, k, :],
                        start=(k == 0),
                        stop=(k == KC - 1),
                    )
                vd = v_sb[tt][:]
                nc.vector.tensor_copy(
                    vd[:, b, :, 0:64], ps[:].rearrange("p (h s) -> p h s", s=64)
                )
                nc.vector.memset(vd[:, b, :, 64:65], 1.0)
            qk_mm(b, xk, 1)
            qk_mm(b, xk, 3)

        def attention(b, ib, hp):
            qt = qk_sb[b][hp]
            kt = qk_sb[b][2 + hp]
            ests = []
            for j in range(TT):
                ps = psS.tile([128, 2, NBLK], FP32, tag="st")
                for h2 in range(2):
                    po = h2 * 64
                    nc.tensor.matmul(
                        ps[:, h2, :],
                        lhsT=kt[po:po + 64, j * 128:(j + 1) * 128],
                        rhs=qt[po:po + 64, ib * NBLK:(ib + 1) * NBLK],
                        start=True,
                        stop=True,
                    )
                e = epool.tile([128, 2, NBLK], BF16, tag="est")
                nc.scalar.activation(e[:], ps[:], EXPF, scale=SCALE)
                ests.append(e)
            for h2 in range(2):
                h = hp * 2 + h2
                av = psAv.tile([128, NBLK], FP32, tag="av")
                for j in range(TT):
                    nc.tensor.matmul(
                        av[0:65, :],
                        lhsT=v_sb[j][:, b, h, :],
                        rhs=ests[j][:, h2, :],
                        start=(j == 0),
                        stop=(j == TT - 1),
                    )
                # custom-DVE recip mis-reads PSUM; stage via SBUF
                rsum = small.tile([1, NBLK], FP32, tag="rsum")
                nc.vector.tensor_copy(rsum[:], av[64:65, :])
                rcp = small.tile([1, NBLK], FP32, tag="rcp")
                nc.vector.reciprocal_approx_fast(rcp[:], rsum[:])
                rbc = small.tile([64, NBLK], FP32, tag="rbc")
                nc.gpsimd.partition_broadcast(rbc[:], rcp[:], channels=64)
                nc.vector.tensor_mul(
                    att_sb[b][hp][h2 * 64:(h2 + 1) * 64,
                                  ib * NBLK:(ib + 1) * NBLK],
                    av[0:64, :],
                    rbc[:],
                )

        def proj(b, ib):
            # stage ct-tiles; identity-mapped DMA per half so the last
            # half's DMA is the only exposed tail
            ot = outs_pool.tile([128, KC, NBLK], BF16, tag="ot")
            for ct in range(KC):
                ps = psA.tile([128, NBLK], FP32, tag="mm")
                for k in range(2):
                    nc.tensor.matmul(
                        ps[:],
                        lhsT=wp_sb[k][:, ct * 128:(ct + 1) * 128],
                        rhs=att_sb[b][k][:, ib * NBLK:(ib + 1) * NBLK],
                        start=(k == 0),
                        stop=(k == 1),
                    )
                nc.vector.tensor_copy(ot[:, ct, :], ps[:])
                if ct == KC // 2 - 1:
                    nc.sync.dma_start(
                        outp[b, ib, :, 0:KC // 2, :], ot[:, 0:KC // 2, :]
                    )
            nc.sync.dma_start(
                outp[b, ib, :, KC // 2:KC, :], ot[:, KC // 2:KC, :]
            )

        phase_a(0, xk0)
        for b in range(NB):
            if b + 1 < NB:
                xk_next = dma_x(b + 1)
            for ib in range(IB):
                for hp in range(NH // 2):
                    attention(b, ib, hp)
                if ib == 0:
                    proj(b, 0)
            # phase A of b+1 fills batch b's ACT-bound PE gaps and keeps the
            # PE warm across the batch boundary; proj(b, 1) slots in behind
            if b + 1 < NB:
                phase_a(b + 1, xk_next)
            proj(b, 1)

    nc.compile()
    return nc


_NC = None


def _get_program():
    global _NC
    if _NC is None:
        _NC = build_program()
    return _NC


def make_in_maps(x, W_qkv):
    """Host-side sharding: per-core input dicts (all bf16, identity layouts)."""
    x = np.asarray(x, dtype=np.float32)
    W_qkv = np.asarray(W_qkv, dtype=np.float32)
    in_maps = []
    for c in range(N_CORES):
        bg, hg = divmod(c, 4)
        # [NB, 128, KC, N]: xb[b, p, k, n] = x[b, n, k*128+p]
        xT_c = x[bg * NB:(bg + 1) * NB].transpose(0, 2, 1)   # [NB, C, N]
        xb_c = np.ascontiguousarray(
            xT_c.reshape(NB, KC, 128, N).transpose(0, 2, 1, 3)
        ).astype(ml_dtypes.bfloat16)
        q_cols = W_qkv[:, hg * NH * D:(hg + 1) * NH * D]
        k_cols = W_qkv[:, H * D + hg * NH * D:H * D + (hg + 1) * NH * D]
        wqk_c = np.concatenate([q_cols, k_cols], axis=1)     # [C, 512]
        # [128, 4, KC, 128]: wqk[p, m, k, c] = wqk_c[k*128+p, m*128+c]
        wqk_c = np.ascontiguousarray(
            wqk_c.reshape(KC, 128, 4, 128).transpose(1, 2, 0, 3)
        ).astype(ml_dtypes.bfloat16)
        wv_c = W_qkv[:, 2 * H * D + hg * NH * D:2 * H * D + (hg + 1) * NH * D]
        wv_c = np.ascontiguousarray(
            wv_c.reshape(KC, 128, NH * D).transpose(1, 0, 2)
        ).astype(ml_dtypes.bfloat16)
        in_maps.append({
            "xb": xb_c,
            "wqk": wqk_c,
            "wv": wv_c,
        })
    return in_maps


def reg_row_constant(x, W_qkv, reg, W_proj):
    """Contribution of the post-softmax reg bias to the output.

    Uniform reg (the generated input): rank-1 per batch -> [B, 1, C].
    Non-uniform reg: full numpy fallback -> [B, N, C].
    """
    x = np.asarray(x, dtype=np.float32)
    W_qkv = np.asarray(W_qkv, dtype=np.float32)
    W_proj = np.asarray(W_proj, dtype=np.float32)
    reg = np.asarray(reg, dtype=np.float32)
    W_v = W_qkv[:, 2 * H * D:]
    if reg.max() == reg.min():
        r = float(reg.flat[0])
        xsum = x.sum(axis=1)                      # [B, C]
        return (r * (xsum @ W_v) @ W_proj)[:, None, :]
    v = (x @ W_v).reshape(B, N, H, D)
    regv = np.einsum("hnm,bmhd->bnhd", reg[0], v)
    return regv.reshape(B, N, H * D) @ W_proj


def assemble_output(results, corr, b_proj):
    b_proj = np.asarray(b_proj, dtype=np.float32)
    out = np.empty((B, N, C), dtype=np.float32)
    for b in range(B):
        bg, bl = divmod(b, NB)
        accT = None
        for hg in range(4):
            o = np.asarray(results[bg * 4 + hg]["outp"][bl])  # [IB,128,KC,NBLK]
            oT = o.transpose(2, 1, 0, 3).reshape(C, N).astype(np.float32)
            accT = oT if accT is None else accT + oT
        cb = corr[b] if corr.shape[1] > 1 else corr[b, 0]
        out[b] = accT.T + (cb + b_proj)
    return out


def kernel(x, W_qkv, reg, W_proj, b_proj, trace=None):
    if trace is None:
        trace = bool(int(os.environ.get("KERNEL_TRACE", "0")))
    nc = _get_program()
    in_maps = make_in_maps(x, W_qkv)
    W_proj = np.asarray(W_proj, dtype=np.float32)
    for c in range(N_CORES):
        hg = c % 4
        in_maps[c]["wp"] = np.ascontiguousarray(
            W_proj[hg * NH * D:(hg + 1) * NH * D, :].reshape(2, 128, C)
        ).astype(ml_dtypes.bfloat16)
    corr = reg_row_constant(x, W_qkv, reg, W_proj)
    try:
        res = run_bass_kernel_spmd(
            nc, in_maps, core_ids=list(range(N_CORES)), trace=trace
        )
    except ModuleNotFoundError:
        # profiling hook unavailable in this environment: run without trace
        res = run_bass_kernel_spmd(
            nc, in_maps, core_ids=list(range(N_CORES)), trace=False
        )
    kernel.last_results = res
    return assemble_output(res.results, corr, b_proj)


# revision 38
# speedup vs baseline: 1.0924x; 1.0047x over previous
"""BottAttention Trainium2 kernel (v4).

Reference computation (per batch b):
    qkv = x @ W_qkv                       # [N, 3*H*D]
    q,k,v per head h (D=64)
    S = q @ k.T * D**-0.5                 # [N, N]
    P = softmax(S, axis=-1) + reg[h]      # post-softmax learned bias
    o = P @ v                             # [N, D]
    out = concat_h(o) @ W_proj + b_proj   # [N, C]

reg as generated is UNIFORM (ones/N): its contribution to out is the
per-batch row-constant  c[b] = r * (x[b].sum(0) @ W_v) @ W_proj,  added
on the host (verified at runtime; general numpy fallback otherwise).
The device computes pure softmax attention, all-bf16 storage, fp32 PSUM.

Sharding: 2 batch-groups x 4 head-groups over 8 cores.  Core c handles
batches 4*(c//4)..+4 and heads 4*(c%4)..+4; host sums the 4 head-group
partial projections, transposes, adds b_proj + c[b].

Device dataflow per (core, batch):
    qT,kT  = (W_qk tiles).T @ xT          # [wcol, tok]
    v      = (xT tiles).T @ W_v           # [tok, vcol] (+ ones col)
    ST     = kT.T @ qT                    # [j, i]; 2 heads concurrently in
                                          # disjoint PE row groups (0/64)
    est    = exp(ST * scale)              # ACT, psum -> sbuf bf16
    avT    = [v | 1].T @ est              # rows 0-63 (attn@v).T, row 64 rowsum
    attnT  = avT * bcast(1/rowsum)        # fast recip + gpsimd bcast
    outT   = W_proj_rows.T @ attnT        # partial [C, i], bf16 out

All DRAM<->SBUF layouts are identity-mapped to the SBUF tiles (multi-KB
contiguous runs per partition, one DMA per tensor/batch).  Emission
pipelines batches: phase A of b+1 is emitted between attention(b) and
proj(b, 1), filling the ACT-bound softmax stretch and keeping the PE warm
across batch boundaries.
"""

import os
import numpy as np
import ml_dtypes
from contextlib import ExitStack

import concourse.bass as bass
import concourse.bacc as bacc
import concourse.tile as tile
from concourse import mybir
from concourse.bass_utils import run_bass_kernel_spmd

FP32 = mybir.dt.float32
BF16 = mybir.dt.bfloat16

# Problem dims (hardcoded per contest contract)
B, H, N, C, D = 8, 16, 1024, 1024, 64
SCALE = D ** -0.5

# Per-core shard dims
NB = 4          # batches per core
NH = 4          # heads per core
KC = C // 128   # contraction tiles for dim C
TT = N // 128   # token tiles
NBLK = 512      # matmul moving-dim block
IB = N // NBLK  # i blocks

N_CORES = 8


def build_program():
    nc = bacc.Bacc("TRN2", debug=False, enable_asserts=False, num_devices=1)

    # identity-mapped layouts: [128 partitions, ...contiguous free dims]
    xb = nc.dram_tensor("xb", [NB, 128, KC, N], BF16, kind="ExternalInput").ap()
    wqk = nc.dram_tensor("wqk", [128, 4, KC, 128], BF16, kind="ExternalInput").ap()
    wv = nc.dram_tensor("wv", [128, KC, NH * D], BF16, kind="ExternalInput").ap()
    wp = nc.dram_tensor("wp", [2, 128, C], BF16, kind="ExternalInput").ap()
    outp = nc.dram_tensor("outp", [NB, IB, 128, KC, NBLK], BF16,
                          kind="ExternalOutput").ap()

    EXPF = mybir.ActivationFunctionType.Exp

    with (
        nc.allow_low_precision(reason="bf16 attention path is intentional"),
        tile.TileContext(nc) as tc,
        ExitStack() as top,
    ):
        # ---- input DMAs first (x batch 0, then weights, in need order) ----
        xpool = top.enter_context(tc.tile_pool(name="xt", bufs=2))
        wpool = top.enter_context(tc.tile_pool(name="wq", bufs=1))

        def dma_x(b, split=False):
            t = xpool.tile([128, KC, N], BF16, tag="xt")
            if split:
                # two halves so batch 0's first qk matmuls start sooner
                nc.sync.dma_start(t[:, 0:KC // 2, :], xb[b, :, 0:KC // 2, :])
                nc.sync.dma_start(t[:, KC // 2:KC, :], xb[b, :, KC // 2:KC, :])
            else:
                nc.sync.dma_start(t[:], xb[b])
            return t

        # first qk group needs wqk[m=0] + x k-tiles: wqk via the scalar
        # HWDGE (in m-need order: 0, 2, 1, 3), x via sync, so the two
        # transfer in parallel right after the framework preamble
        wqk_sb = wpool.tile([128, 4, KC, 128], BF16, tag="wqk")
        nc.scalar.dma_start(wqk_sb[:, 0], wqk[:, 0])
        xk0 = dma_x(0, split=True)
        nc.scalar.dma_start(wqk_sb[:, 2], wqk[:, 2])
        nc.scalar.dma_start(wqk_sb[:, 1], wqk[:, 1])
        nc.scalar.dma_start(wqk_sb[:, 3], wqk[:, 3])
        wv_sb = wpool.tile([128, KC, NH * D], BF16, tag="wv")
        nc.scalar.dma_start(wv_sb[:], wv)
        persist = top.enter_context(tc.tile_pool(name="persist", bufs=1))
        wp_sb = [persist.tile([128, C], BF16, tag=f"wp{k}", name=f"wp{k}")
                 for k in range(2)]
        for k in range(2):
            nc.sync.dma_start(wp_sb[k][:], wp[k])

        # qk_sb[b][m]: m=0,1 -> qT head pairs; m=2,3 -> kT pairs
        qk_sb = [
            [persist.tile([128, N], BF16, tag=f"qk{b}_{m}", name=f"qk{b}_{m}")
             for m in range(4)]
            for b in range(NB)
        ]
        # v_sb[t]: [128, NB, NH, 65] bf16; 64 v cols + ones col (rowsum)
        v_sb = [
            persist.tile([128, NB, NH, 65], BF16, tag=f"v{t}", name=f"v{t}")
            for t in range(TT)
        ]
        # att_sb[b][hp]: normalized (attn@v).T for head pair hp
        att_sb = [
            [persist.tile([128, N], BF16, tag=f"at{b}_{k}", name=f"at{b}_{k}")
             for k in range(2)]
            for b in range(NB)
        ]

        epool = top.enter_context(tc.tile_pool(name="est", bufs=13))
        small = top.enter_context(tc.tile_pool(name="sm", bufs=2))
        outs_pool = top.enter_context(tc.tile_pool(name="outs", bufs=2))
        psA = top.enter_context(tc.tile_pool(name="psA", bufs=2, space="PSUM"))
        psS = top.enter_context(tc.tile_pool(name="psS", bufs=2, space="PSUM"))
        psAv = top.enter_context(tc.tile_pool(name="psAv", bufs=2, space="PSUM"))

        # PE warmup: cheap matmuls so HAM un-throttles while x DMA lands
        warm_src = wpool.tile([1, NBLK], BF16, tag="warm_src")
        nc.vector.memset(warm_src[:], 1.0)
        wps = psA.tile([1, NBLK], FP32, tag="mm")
        for w in range(6):
            nc.tensor.matmul(
                wps[:], lhsT=warm_src[0:1, 0:1], rhs=warm_src[:],
                start=True, stop=True,
            )

        def qk_mm(b, xk, m):
            for tb in range(IB):
                ps = psA.tile([128, NBLK], FP32, tag="mm")
                for k in range(KC):
                    nc.tensor.matmul(
                        ps[:],
                        lhsT=wqk_sb[:, m, k, :],
                        rhs=xk[:, k, tb * NBLK:(tb + 1) * NBLK],
                        start=(k == 0),
                        stop=(k == KC - 1),
                    )
                nc.any.tensor_copy(
                    qk_sb[b][m][:, tb * NBLK:(tb + 1) * NBLK], ps[:]
                )

        def phase_a(b, xk):
            # hp0's q/k first, then v, then hp1's: attention(b, 0, 0) can
            # start after m=0,2 and v
            qk_mm(b, xk, 0)
            qk_mm(b, xk, 2)
            for tt in range(TT):
                ps = psA.tile([128, NH * D], FP32, tag="mm")
                for k in range(KC):
                    nc.tensor.matmul(
                        ps[:],
                        lhsT=xk[:, k, tt * 128:(tt + 1) * 128],
                        rhs=wv_sb[:, k, :],
                        start=(k == 0),
                        stop=(k == KC - 1),
                    )
                vd = v_sb[tt][:]
                nc.any.tensor_copy(
                    vd[:, b, :, 0:64], ps[:].rearrange("p (h s) -> p h s", s=64)
                )
                nc.vector.memset(vd[:, b, :, 64:65], 1.0)
            qk_mm(b, xk, 1)
            qk_mm(b, xk, 3)

        def attention(b, ib, hp):
            qt = qk_sb[b][hp]
            kt = qk_sb[b][2 + hp]
            ests = []
            for j in range(TT):
                ps = psS.tile([128, 2, NBLK], FP32, tag="st")
                for h2 in range(2):
                    po = h2 * 64
                    nc.tensor.matmul(
                        ps[:, h2, :],
                        lhsT=kt[po:po + 64, j * 128:(j + 1) * 128],
                        rhs=qt[po:po + 64, ib * NBLK:(ib + 1) * NBLK],
                        start=True,
                        stop=True,
                    )
                e = epool.tile([128, 2, NBLK], BF16, tag="est")
                nc.scalar.activation(e[:], ps[:], EXPF, scale=SCALE)
                ests.append(e)
            for h2 in range(2):
                h = hp * 2 + h2
                av = psAv.tile([128, NBLK], FP32, tag="av")
                for j in range(TT):
                    nc.tensor.matmul(
                        av[0:65, :],
                        lhsT=v_sb[j][:, b, h, :],
                        rhs=ests[j][:, h2, :],
                        start=(j == 0),
                        stop=(j == TT - 1),
                    )
                # custom-DVE recip mis-reads PSUM; stage via SBUF
                rsum = small.tile([1, NBLK], FP32, tag="rsum")
                nc.vector.tensor_copy(rsum[:], av[64:65, :])
                rcp = small.tile([1, NBLK], FP32, tag="rcp")
                nc.vector.reciprocal_approx_fast(rcp[:], rsum[:])
                rbc = small.tile([64, NBLK], FP32, tag="rbc")
                nc.gpsimd.partition_broadcast(rbc[:], rcp[:], channels=64)
                nc.vector.tensor_mul(
                    att_sb[b][hp][h2 * 64:(h2 + 1) * 64,
                                  ib * NBLK:(ib + 1) * NBLK],
                    av[0:64, :],
                    rbc[:],
                )

        def proj(b, ib, tail=False):
            # stage ct-tiles; identity-mapped DMA per chunk so only the
            # last chunk's DMA is exposed.  At the kernel tail ACT is idle:
            # alternate the psum-drain copies across vector/scalar and use
            # quarter-DMAs to shrink the exposed end.
            ot = outs_pool.tile([128, KC, NBLK], BF16, tag="ot")
            nchunk = 4 if tail else 2
            cw = KC // nchunk
            for ct in range(KC):
                ps = psA.tile([128, NBLK], FP32, tag="mm")
                for k in range(2):
                    nc.tensor.matmul(
                        ps[:],
                        lhsT=wp_sb[k][:, ct * 128:(ct + 1) * 128],
                        rhs=att_sb[b][k][:, ib * NBLK:(ib + 1) * NBLK],
                        start=(k == 0),
                        stop=(k == 1),
                    )
                if tail and ct % 2 == 1:
                    nc.scalar.copy(ot[:, ct, :], ps[:])
                else:
                    nc.vector.tensor_copy(ot[:, ct, :], ps[:])
                if (ct + 1) % cw == 0 and ct + 1 < KC:
                    c0 = ct + 1 - cw
                    nc.sync.dma_start(
                        outp[b, ib, :, c0:ct + 1, :], ot[:, c0:ct + 1, :]
                    )
            nc.sync.dma_start(
                outp[b, ib, :, KC - cw:KC, :], ot[:, KC - cw:KC, :]
            )

        phase_a(0, xk0)
        for b in range(NB):
            if b + 1 < NB:
                xk_next = dma_x(b + 1)
            for ib in range(IB):
                for hp in range(NH // 2):
                    attention(b, ib, hp)
                if ib == 0:
                    proj(b, 0)
            # phase A of b+1 fills batch b's ACT-bound PE gaps and keeps the
            # PE warm across the batch boundary; proj(b, 1) slots in behind
            if b + 1 < NB:
                phase_a(b + 1, xk_next)
            proj(b, 1, tail=(b == NB - 1))

    nc.compile()
    return nc


_NC = None


def _get_program():
    global _NC
    if _NC is None:
        _NC = build_program()
    return _NC


def make_in_maps(x, W_qkv):
    """Host-side sharding: per-core input dicts (all bf16, identity layouts)."""
    x = np.asarray(x, dtype=np.float32)
    W_qkv = np.asarray(W_qkv, dtype=np.float32)
    in_maps = []
    for c in range(N_CORES):
        bg, hg = divmod(c, 4)
        # [NB, 128, KC, N]: xb[b, p, k, n] = x[b, n, k*128+p]
        xT_c = x[bg * NB:(bg + 1) * NB].transpose(0, 2, 1)   # [NB, C, N]
        xb_c = np.ascontiguousarray(
            xT_c.reshape(NB, KC, 128, N).transpose(0, 2, 1, 3)
        ).astype(ml_dtypes.bfloat16)
        q_cols = W_qkv[:, hg * NH * D:(hg + 1) * NH * D]
        k_cols = W_qkv[:, H * D + hg * NH * D:H * D + (hg + 1) * NH * D]
        wqk_c = np.concatenate([q_cols, k_cols], axis=1)     # [C, 512]
        # [128, 4, KC, 128]: wqk[p, m, k, c] = wqk_c[k*128+p, m*128+c]
        wqk_c = np.ascontiguousarray(
            wqk_c.reshape(KC, 128, 4, 128).transpose(1, 2, 0, 3)
        ).astype(ml_dtypes.bfloat16)
        wv_c = W_qkv[:, 2 * H * D + hg * NH * D:2 * H * D + (hg + 1) * NH * D]
        wv_c = np.ascontiguousarray(
            wv_c.reshape(KC, 128, NH * D).transpose(1, 0, 2)
        ).astype(ml_dtypes.bfloat16)
        in_maps.append({
            "xb": xb_c,
            "wqk": wqk_c,
            "wv": wv_c,
        })
    return in_maps


def reg_row_constant(x, W_qkv, reg, W_proj):
    """Contribution of the post-softmax reg bias to the output.

    Uniform reg (the generated input): rank-1 per batch -> [B, 1, C].
    Non-uniform reg: full numpy fallback -> [B, N, C].
    """
    x = np.asarray(x, dtype=np.float32)
    W_qkv = np.asarray(W_qkv, dtype=np.float32)
    W_proj = np.asarray(W_proj, dtype=np.float32)
    reg = np.asarray(reg, dtype=np.float32)
    W_v = W_qkv[:, 2 * H * D:]
    if reg.max() == reg.min():
        r = float(reg.flat[0])
        xsum = x.sum(axis=1)                      # [B, C]
        return (r * (xsum @ W_v) @ W_proj)[:, None, :]
    v = (x @ W_v).reshape(B, N, H, D)
    regv = np.einsum("hnm,bmhd->bnhd", reg[0], v)
    return regv.reshape(B, N, H * D) @ W_proj


def assemble_output(results, corr, b_proj):
    b_proj = np.asarray(b_proj, dtype=np.float32)
    out = np.empty((B, N, C), dtype=np.float32)
    for b in range(B):
        bg, bl = divmod(b, NB)
        accT = None
        for hg in range(4):
            o = np.asarray(results[bg * 4 + hg]["outp"][bl])  # [IB,128,KC,NBLK]
            oT = o.transpose(2, 1, 0, 3).reshape(C, N).astype(np.float32)
            accT = oT if accT is None else accT + oT
        cb = corr[b] if corr.shape[1] > 1 else corr[b, 0]
        out[b] = accT.T + (cb + b_proj)
    return out


def kernel(x, W_qkv, reg, W_proj, b_proj, trace=None):
    if trace is None:
        trace = bool(int(os.environ.get("KERNEL_TRACE", "0")))
    nc = _get_program()
    in_maps = make_in_maps(x, W_qkv)
    W_proj = np.asarray(W_proj, dtype=np.float32)
    for c in range(N_CORES):
        hg = c % 4
        in_maps[c]["wp"] = np.ascontiguousarray(
            W_proj[hg * NH * D:(hg + 1) * NH * D, :].reshape(2, 128, C)
        ).astype(ml_dtypes.bfloat16)
    corr = reg_row_constant(x, W_qkv, reg, W_proj)
    try:
        res = run_bass_kernel_spmd(
            nc, in_maps, core_ids=list(range(N_CORES)), trace=trace
        )
    except ModuleNotFoundError:
        # profiling hook unavailable in this environment: run without trace
        res = run_bass_kernel_spmd(
            nc, in_maps, core_ids=list(range(N_CORES)), trace=False
        )
    kernel.last_results = res
    return assemble_output(res.results, corr, b_proj)
